# revision 48
# baseline (speedup 1.0000x reference)
"""Trainium2 Bass kernel for nn_EventEncoder (2-layer varlen-packed transformer).

Strategy: sequence-parallel over 8 NeuronCores (512 tokens = 16 whole events per
core, block-diagonal causal attention => no cross-core communication). Weights
replicated, streamed from HBM in bf16. Activations feature-major [D, T].

This revision restructures the schedule around the engine cost model:
 - all hot DVE elementwise ops use bf16 operands in SBUF (4x DVE mode)
 - rmsnorm is deferred: projections read a plain bf16 cast of h; the 1/rms
   factor is folded into the rope tables (q,k) and the V-output scale, so the
   PE never waits on the rmsnorm stat chain in the attention path
 - softmax 1/den via vector.reciprocal; silu via ACT Silu; Sqrt-based rsqrt
   => ~4 ACT table loads per layer instead of ~18
 - rmsnorm/final-norm stats interleave with the residual-add loops
 - embedding gather DMAs are issued first and the transpose/cast pipeline
   follows per 128-token tile

Self-contained: hardcodes all shapes from the problem spec.
"""
import sys
sys.path.insert(0, "/opt/trn_rl_repo")

import numpy as np
import ml_dtypes
from contextlib import ExitStack

import concourse.bass as bass
import concourse.tile as tile
from concourse import bacc, mybir
from concourse.masks import make_identity

# ---- problem constants (hardcoded from spec) ----
S = 4096
NSEG = 128
EVLEN = 32
MSL = 16          # max_seq_len (events per user)
VOCAB = 32002
D = 1024
H = 8
DH = 128
DFF = 4096
L = 2
ROPE_BASE = 10000.0

NCORES = 8
T = S // NCORES       # 512 tokens per core
TT = T // 128         # 4 token tiles
KD = D // 128         # 8 feature tiles
KF = DFF // 128       # 32 ffn tiles
SCALE = 1.0 / float(np.sqrt(DH))

F32 = mybir.dt.float32
BF16 = mybir.dt.bfloat16
I32 = mybir.dt.int32
AF = mybir.ActivationFunctionType
ALU = mybir.AluOpType

MMDT = BF16
MMNP = ml_dtypes.bfloat16


# =============================================================
# device program
# =============================================================

def build_program():
    nc = bacc.Bacc("TRN2", target_bir_lowering=False, debug=False)

    dt_w = MMDT
    # ---- inputs ----
    emb_d = nc.dram_tensor("emb", [VOCAB, D], F32, kind="ExternalInput").ap()
    ids_d = nc.dram_tensor("idsc", [TT, 128, 1], I32, kind="ExternalInput").ap()
    mask_d = nc.dram_tensor("maskT", [TT, 128, 128], MMDT, kind="ExternalInput").ap()
    lnf_d = nc.dram_tensor("lnft", [KD, 128], F32, kind="ExternalInput").ap()
    wq_d = nc.dram_tensor("wq", [L, KD, KD, 128, 128], dt_w, kind="ExternalInput").ap()
    wk_d = nc.dram_tensor("wk", [L, KD, KD, 128, 128], dt_w, kind="ExternalInput").ap()
    wv_d = nc.dram_tensor("wv", [L, KD, 2, 128, 512], dt_w, kind="ExternalInput").ap()
    wo_d = nc.dram_tensor("wo", [L, KD, KD, 128, 128], dt_w, kind="ExternalInput").ap()
    w1_d = nc.dram_tensor("w1", [L, KF, KD, 128, 128], dt_w, kind="ExternalInput").ap()
    w2_d = nc.dram_tensor("w2", [L, KD, KF, 128, 128], dt_w, kind="ExternalInput").ap()
    cs_d = nc.dram_tensor("costab", [128, T], MMDT, kind="ExternalInput").ap()
    sn_d = nc.dram_tensor("sintab", [128, T], MMDT, kind="ExternalInput").ap()

    out_d = nc.dram_tensor("out", [KD, 128, MSL], F32, kind="ExternalOutput").ap()

    with tile.TileContext(nc) as tc, ExitStack() as ctx:
        persist = ctx.enter_context(tc.tile_pool(name="persist", bufs=1))
        acts = ctx.enter_context(tc.tile_pool(name="acts", bufs=1))
        wpool = ctx.enter_context(tc.tile_pool(name="wpool", bufs=6))
        w2pool = ctx.enter_context(tc.tile_pool(name="w2pool", bufs=3))
        tmp = ctx.enter_context(tc.tile_pool(name="tmp", bufs=3))
        eodp = ctx.enter_context(tc.tile_pool(name="eodp", bufs=3))
        epool = ctx.enter_context(tc.tile_pool(name="epool", bufs=6))
        dinvp = ctx.enter_context(tc.tile_pool(name="dinvp", bufs=4))
        dbcp = ctx.enter_context(tc.tile_pool(name="dbcp", bufs=3))
        wqpre = ctx.enter_context(tc.tile_pool(name="wqpre", bufs=2))
        gpool = ctx.enter_context(tc.tile_pool(name="gpool", bufs=2))
        rowp = ctx.enter_context(tc.tile_pool(name="rowp", bufs=1))
        ps_mm = ctx.enter_context(tc.tile_pool(name="ps_mm", bufs=4, space="PSUM"))
        ps_att = ctx.enter_context(tc.tile_pool(name="ps_att", bufs=2, space="PSUM"))
        ps_row = ps_att  # row-psums ([1, n]) borrow the att_o slots

        # ---------- embedding gather issued FIRST ----------
        g_tiles = []
        for t in range(TT):
            ids_sb = dinvp.tile([128, 1], I32, tag="ids_sb")
            nc.sync.dma_start(out=ids_sb, in_=ids_d[t])
            g = gpool.tile([128, D], F32, tag="g")
            nc.gpsimd.indirect_dma_start(
                out=g[:], out_offset=None, in_=emb_d[:],
                in_offset=bass.IndirectOffsetOnAxis(ap=ids_sb[:, 0:1], axis=0),
            )
            g_tiles.append(g)

        # ---------- persistent tiles / other DMAs (overlap the gather) ----------
        hT = persist.tile([128, KD, T], F32, tag="hT")
        h16 = persist.tile([128, KD, T], MMDT, tag="h16")
        ident = persist.tile([128, 128], F32, tag="ident")
        make_identity(nc, ident)
        oneD_col = persist.tile([128, 1], MMDT, tag="oneD_col")  # 1/D for ssq
        nc.vector.memset(oneD_col, 1.0 / D)
        ones_col = persist.tile([128, 1], MMDT, tag="ones_col")  # 1.0 for den
        nc.vector.memset(ones_col, 1.0)
        ones_row = persist.tile([1, 128], MMDT, tag="ones_row")  # K=1 -> bcast
        nc.vector.memset(ones_row, 1.0)
        ones11 = persist.tile([1, 1], MMDT, tag="ones11")
        nc.vector.memset(ones11, 1.0)
        warm_in = persist.tile([1, 1], F32, tag="warm_in")
        nc.vector.memset(warm_in, 1.0)
        warm_out = persist.tile([1, 1], F32, tag="warm_out")

        def act_warm(func, anchor):
            # dummy activation anchored on `anchor` (a [1,1] AP): pins the ACT
            # table load into an idle window instead of the critical chain
            nc.scalar.activation(out=warm_out, in_=anchor, func=func)
        mask_sb = persist.tile([128, TT, 128], MMDT, tag="mask_sb")
        nc.sync.dma_start(out=mask_sb, in_=mask_d.transpose([1, 0, 2]))
        mask_flat = mask_sb.rearrange("p t q -> p (t q)")
        lnf_sb = persist.tile([128, KD], F32, tag="lnf_sb")
        nc.sync.dma_start(out=lnf_sb, in_=lnf_d.transpose([1, 0]))
        cos16 = persist.tile([128, T], MMDT, tag="cos16")
        sin16 = persist.tile([128, T], MMDT, tag="sin16")
        nc.sync.dma_start(out=cos16, in_=cs_d)
        nc.sync.dma_start(out=sin16, in_=sn_d)

        def prefetch_wq(l_):
            wg_e = wqpre.tile([128, KD, 128], dt_w, tag="wqpre")
            nc.sync.dma_start(out=wg_e, in_=wq_d[l_, 0].transpose([1, 0, 2]))
            wg_o = wqpre.tile([128, KD, 128], dt_w, tag="wqpre")
            nc.sync.dma_start(out=wg_o, in_=wq_d[l_, 4].transpose([1, 0, 2]))
            return wg_e, wg_o

        # wv for layer 0 streams during the gather (fine-grained for DMA-queue
        # parallelism; layer-0 V projections need it ~15us in)
        wv_sb = acts.tile([128, KD, 2, 512], dt_w, tag="wv_sb")
        for nh_ in range(2):
            for q in range(4):
                hs = slice(q * 2, q * 2 + 2)
                nc.sync.dma_start(out=wv_sb[:, hs, nh_, :],
                                  in_=wv_d[0, hs, nh_].transpose([1, 0, 2]))
        v_sb = acts.tile([128, TT, 2, 512], MMDT, tag="v_sb")
        pre_q = prefetch_wq(0)

        # ---------- gather -> transpose -> hT/h16; layer-0 V projections
        # interleave per token tile to fill the PE during the gather tail
        # (the rinv scale is applied to v_sb in place once stats are ready) ----
        for t in range(TT):
            for d in range(KD):
                tp_ps = ps_mm.tile([128, 128], F32, tag="mm512")
                nc.tensor.transpose(out=tp_ps, in_=g_tiles[t][:, d * 128:(d + 1) * 128],
                                    identity=ident)
                cs = slice(t * 128, (t + 1) * 128)
                nc.vector.tensor_copy(out=hT[:, d, cs], in_=tp_ps)
                nc.scalar.copy(out=h16[:, d, cs], in_=tp_ps)
                if t == 0 and d == 0:
                    act_warm(AF.Sqrt, h16[0:1, 0, 0:1])
            for nh_v in range(2):
                v_ps = ps_mm.tile([128, 512], F32, tag="mm512")
                for kt in range(KD):
                    nc.tensor.matmul(
                        v_ps,
                        h16[:, kt, t * 128:(t + 1) * 128],
                        wv_sb[:, kt, nh_v, :],
                        start=(kt == 0), stop=(kt == KD - 1))
                nc.scalar.copy(out=v_sb[:, t, nh_v, :], in_=v_ps)

        # ---------- helpers ----------
        def rms_stats(sq_src, n_free, tag):
            """sq_src: list of KD bf16 sq tiles [128, n_free] (already squared).
            Returns rinv16 [1, n_free] bf16 = rsqrt(mean + eps)."""
            ssq_ps = ps_row.tile([1, n_free], F32, tag="att_o")
            for d in range(KD):
                nc.tensor.matmul(ssq_ps, oneD_col, sq_src[d],
                                 start=(d == 0), stop=(d == KD - 1))
            return rms_finish(ssq_ps, n_free, tag, eps=True)

        def rms_finish(ssq_ps, n_free, tag, eps=False):
            # eps=1e-6 matters only at the embedding scale (layer-0 rms1);
            # everywhere else mean-sq >= ~0.2 and the add is skipped.
            src_ap = ssq_ps
            if eps:
                m_eps = rowp.tile([1, n_free], F32, tag="me")
                nc.vector.tensor_scalar_add(m_eps, ssq_ps, 1e-6)
                src_ap = m_eps
            rec = rowp.tile([1, n_free], F32, tag="rc")
            nc.vector.reciprocal_approx_fast(out=rec, in_=src_ap)
            rinv16 = rowp.tile([1, n_free], MMDT, tag="ri")
            nc.scalar.activation(out=rinv16, in_=rec, func=AF.Sqrt)
            return rinv16

        def bcast16(rinv16, n_free, tag):
            """broadcast [1, n] bf16 row to [128, n] bf16 SBUF tile."""
            bc_ps = ps_mm.tile([128, n_free], F32, tag="mm512")
            nc.tensor.matmul(bc_ps, ones_row, rinv16, start=True, stop=True)
            bc = acts.tile([128, n_free], MMDT, tag=f"bc_{tag}")
            nc.scalar.copy(out=bc, in_=bc_ps)
            return bc

        # ---------- layer-0 rmsnorm stats (pipelined after casts) ----------
        sqs = []
        for d in range(KD):
            sq = tmp.tile([128, T], MMDT, tag="sq")
            nc.vector.tensor_mul(sq, h16[:, d, :], h16[:, d, :])
            sqs.append(sq)
        rinv16 = rms_stats(sqs, T, "rms1")

        # ---------- layers ----------
        for l in range(L):
            # ---- Q, K projections (read h16 directly) + rope (bf16, 4x DVE) ----
            qrot = acts.tile([128, KD, T], MMDT, tag="qrot")
            krot = acts.tile([128, KD, T], MMDT, tag="krot")
            cosL = acts.tile([128, T], MMDT, tag="cosL")
            sinL = acts.tile([128, T], MMDT, tag="sinL")
            rinv_col = persist.tile([128, TT], F32, tag=f"rcol{l}")

            def qk_group(w_d_, pair, pre=None):
                if pre is not None:
                    wg_e, wg_o = pre
                else:
                    wg_e = wpool.tile([128, KD, 128], dt_w, tag="wtile")
                    nc.sync.dma_start(out=wg_e, in_=w_d_[l, pair].transpose([1, 0, 2]))
                    wg_o = wpool.tile([128, KD, 128], dt_w, tag="wtile")
                    nc.sync.dma_start(out=wg_o, in_=w_d_[l, pair + 4].transpose([1, 0, 2]))
                ev_ps = ps_mm.tile([128, T], F32, tag="mm512")
                od_ps = ps_mm.tile([128, T], F32, tag="mm512")
                for kt in range(KD):
                    nc.tensor.matmul(ev_ps, wg_e[:, kt, :], h16[:, kt, :],
                                     start=(kt == 0), stop=(kt == KD - 1))
                for kt in range(KD):
                    nc.tensor.matmul(od_ps, wg_o[:, kt, :], h16[:, kt, :],
                                     start=(kt == 0), stop=(kt == KD - 1))
                return ev_ps, od_ps

            def rope_apply(rot, pair, ev_ps, od_ps):
                ev16 = eodp.tile([128, T], MMDT, tag="eod")
                od16 = eodp.tile([128, T], MMDT, tag="eod")
                nc.scalar.copy(out=ev16, in_=ev_ps)
                nc.scalar.copy(out=od16, in_=od_ps)
                t1 = tmp.tile([128, T], MMDT, tag="rtmp")
                t2 = tmp.tile([128, T], MMDT, tag="rtmp")
                nc.vector.tensor_mul(t1, ev16, cosL)
                nc.vector.tensor_mul(t2, od16, sinL)
                nc.vector.tensor_sub(rot[:, pair, :], t1, t2)
                t3 = tmp.tile([128, T], MMDT, tag="rtmp")
                t4 = tmp.tile([128, T], MMDT, tag="rtmp")
                nc.vector.tensor_mul(t3, ev16, sinL)
                nc.vector.tensor_mul(t4, od16, cosL)
                nc.vector.tensor_add(rot[:, pair + 4, :], t3, t4)

            # first q group goes ahead of the rinv-dependent table prep so the
            # PE never idles waiting on the rmsnorm chain
            ev0, od0 = qk_group(wq_d, 0, pre=pre_q)
            act_warm(AF.Exp, rinv16[0:1, 0:1])
            bc1 = bcast16(rinv16, T, "r1")
            nc.vector.tensor_mul(cosL, cos16, bc1)
            nc.vector.tensor_mul(sinL, sin16, bc1)
            for t in range(TT):
                rc_ps = ps_row.tile([128, 1], F32, tag="att_o")
                nc.tensor.matmul(rc_ps, rinv16[:, t * 128:(t + 1) * 128], ones11,
                                 start=True, stop=True)
                nc.vector.tensor_copy(out=rinv_col[:, t:t + 1], in_=rc_ps)
            rope_apply(qrot, 0, ev0, od0)
            for pair in range(4):
                for (w_d_, rot) in ((wq_d, qrot), (wk_d, krot)):
                    if pair == 0 and rot is qrot:
                        continue
                    ev_ps, od_ps = qk_group(w_d_, pair)
                    rope_apply(rot, pair, ev_ps, od_ps)

            # ---- V projections (read h16; rinv applied on the psum->sbuf copy).
            # Layer 0's V matmuls already ran in the prologue: only the rinv
            # scale fixup remains (in place).
            if l == 0:
                for t_v in range(TT):
                    for nh_v in range(2):
                        nc.scalar.activation(
                            out=v_sb[:, t_v, nh_v, :], in_=v_sb[:, t_v, nh_v, :],
                            func=AF.Copy, scale=rinv_col[:, t_v:t_v + 1])
            else:
                for t_v in range(TT):
                    for nh_v in range(2):
                        v_ps = ps_mm.tile([128, 512], F32, tag="mm512")
                        for kt in range(KD):
                            nc.tensor.matmul(
                                v_ps,
                                h16[:, kt, t_v * 128:(t_v + 1) * 128],
                                wv_sb[:, kt, nh_v, :],
                                start=(kt == 0), stop=(kt == KD - 1))
                        nc.scalar.activation(out=v_sb[:, t_v, nh_v, :], in_=v_ps,
                                             func=AF.Copy, scale=rinv_col[:, t_v:t_v + 1])

            # ---- attention (software-pipelined: den lags scores by 1 head,
            # dbc + attn@V lag by 2, so the PE never waits on the ACT/DVE
            # softmax chain) ----
            oT = acts.tile([128, KD, T], MMDT, tag="oT")
            e_tiles = {}
            den_tiles = {}
            dbc_tiles = {}

            def scores(h):
                me, mo, off = h // 2, 4 + h // 2, (h % 2) * 64
                s_ps = ps_att.tile([128, T], F32, tag="att_s")
                for t in range(TT):
                    ts_ = slice(t * 128, (t + 1) * 128)
                    nc.tensor.matmul(s_ps[:, ts_], krot[off:off + 64, me, ts_],
                                     qrot[off:off + 64, me, ts_],
                                     start=True, stop=False)
                    nc.tensor.matmul(s_ps[:, ts_], krot[off:off + 64, mo, ts_],
                                     qrot[off:off + 64, mo, ts_],
                                     start=False, stop=True)
                ef16 = tmp.tile([128, T], MMDT, tag="ef16")
                nc.scalar.activation(out=ef16, in_=s_ps, func=AF.Exp, scale=float(SCALE))
                em = epool.tile([128, T], MMDT, tag="e_mm")
                nc.vector.tensor_mul(em, ef16, mask_flat)
                e_tiles[h] = em

            def den(h):
                den_ps = ps_row.tile([1, T], F32, tag="att_o")
                nc.tensor.matmul(den_ps, ones_col, e_tiles[h], start=True, stop=True)
                dinv_f = dinvp.tile([1, T], F32, tag="dinv_f")
                nc.vector.reciprocal_approx_fast(out=dinv_f, in_=den_ps)
                dinv16 = dinvp.tile([1, T], MMDT, tag="dinv16")
                nc.vector.tensor_copy(out=dinv16, in_=dinv_f)
                den_tiles[h] = dinv16

            def attn_v(h):
                dbc_ps = ps_mm.tile([128, T], F32, tag="mm512")
                nc.tensor.matmul(dbc_ps, ones_row, den_tiles[h], start=True, stop=True)
                dbc16 = dbcp.tile([128, T], MMDT, tag="dbc16")
                nc.scalar.copy(out=dbc16, in_=dbc_ps)
                o_ps = ps_att.tile([128, T], F32, tag="att_o")
                for t in range(TT):
                    ts_ = slice(t * 128, (t + 1) * 128)
                    nc.tensor.matmul(
                        o_ps[:, ts_],
                        v_sb[:, t, h // 4, (h % 4) * 128:(h % 4 + 1) * 128],
                        e_tiles[h][:, ts_], start=True, stop=True)
                nc.vector.tensor_mul(oT[:, h, :], o_ps, dbc16)

            for h in range(H):
                scores(h)
                if h >= 2:
                    den(h - 2)
                if h >= 3:
                    attn_v(h - 3)
            den(H - 2)
            den(H - 1)
            attn_v(H - 3)
            attn_v(H - 2)
            attn_v(H - 1)
            act_warm(AF.Sqrt, e_tiles[H - 1][0:1, 0:1])

            # ---- Wo + residual; rmsnorm2 stats interleaved (2-group lag) ----
            sq2s = []
            ssq2_ps = ps_row.tile([1, T], F32, tag="att_o")

            def wo_group(m):
                wg = wpool.tile([128, KD, 128], dt_w, tag="wtile")
                nc.sync.dma_start(out=wg, in_=wo_d[l, m].transpose([1, 0, 2]))
                wo_ps = ps_mm.tile([128, T], F32, tag="mm512")
                for kt in range(KD):
                    nc.tensor.matmul(wo_ps, wg[:, kt, :], oT[:, kt, :],
                                     start=(kt == 0), stop=(kt == KD - 1))
                nc.vector.tensor_add(hT[:, m, :], hT[:, m, :], wo_ps)
                if m < KD - 1:  # last cast deferred past the rms Sqrt (ACT order)
                    nc.scalar.copy(out=h16[:, m, :], in_=hT[:, m, :])
                sq = tmp.tile([128, T], MMDT, tag="sq")
                nc.vector.tensor_mul(sq, hT[:, m, :], hT[:, m, :])
                sq2s.append(sq)

            def ssq2_mm(d):
                nc.tensor.matmul(ssq2_ps, oneD_col, sq2s[d],
                                 start=(d == 0), stop=(d == KD - 1))

            for m in range(KD):
                wo_group(m)
                if m >= 2:
                    ssq2_mm(m - 2)
            ssq2_mm(KD - 2)
            ssq2_mm(KD - 1)
            rinv2 = rms_finish(ssq2_ps, T, "rms2")
            bc2 = bcast16(rinv2, T, "r2")
            act_warm(AF.Silu, bc2[0:1, 0:1])
            nc.scalar.copy(out=h16[:, KD - 1, :], in_=hT[:, KD - 1, :])

            # ---- MLP ----
            x2T = acts.tile([128, KD, T], MMDT, tag="x2T")
            for d in range(KD):
                nc.vector.tensor_mul(x2T[:, d, :], h16[:, d, :], bc2)
            y1 = acts.tile([128, KF, 512], MMDT, tag="y1")
            for j in range(KF):
                wg = wpool.tile([128, KD, 128], dt_w, tag="wtile")
                nc.sync.dma_start(out=wg, in_=w1_d[l, j].transpose([1, 0, 2]))
                y1_ps = ps_mm.tile([128, T], F32, tag="mm512")
                for kt in range(KD):
                    nc.tensor.matmul(y1_ps, wg[:, kt, :], x2T[:, kt, :],
                                     start=(kt == 0), stop=(kt == KD - 1))
                nc.scalar.activation(out=y1[:, j, :], in_=y1_ps, func=AF.Silu)
            act_warm(AF.Sqrt, y1[0:1, KF - 1, 0:1])

            last_layer = (l == L - 1)
            # prefetch next layer's wv + first q-pair weights ahead of the W2
            # DMA storm so the next layer's QK phase starts without a DMA wait
            if not last_layer:
                for nh_ in range(2):
                    nc.sync.dma_start(out=wv_sb[:, :, nh_, :],
                                      in_=wv_d[l + 1, :, nh_].transpose([1, 0, 2]))
                pre_q = prefetch_wq(l + 1)
            sq3s = []
            ssq3_ps = ps_row.tile([1, T if not last_layer else MSL], F32, tag="att_o")
            exts = []

            def w2_group(m):
                wg2 = w2pool.tile([128, KF, 128], dt_w, tag="w2tile")
                for half in range(2):
                    hs = slice(half * (KF // 2), (half + 1) * (KF // 2))
                    nc.sync.dma_start(out=wg2[:, hs, :],
                                      in_=w2_d[l, m, hs].transpose([1, 0, 2]))
                y2_ps = ps_mm.tile([128, T], F32, tag="mm512")
                for j in range(KF):
                    nc.tensor.matmul(y2_ps, wg2[:, j, :], y1[:, j, :],
                                     start=(j == 0), stop=(j == KF - 1))
                nc.vector.tensor_add(hT[:, m, :], hT[:, m, :], y2_ps)
                if not last_layer:
                    nc.scalar.copy(out=h16[:, m, :], in_=hT[:, m, :])
                    sq = tmp.tile([128, T], MMDT, tag="sq")
                    nc.vector.tensor_mul(sq, hT[:, m, :], hT[:, m, :])
                    sq3s.append(sq)
                else:
                    # extract last-token columns, square them (final norm stats)
                    ext = persist.tile([128, MSL], F32, tag=f"ext{m}")
                    src = hT[:, m, :].rearrange("p (e w) -> p e w", w=EVLEN)[:, :, EVLEN - 1]
                    nc.vector.tensor_copy(out=ext, in_=src)
                    exts.append(ext)
                    sq = tmp.tile([128, MSL], MMDT, tag="sqe")
                    nc.vector.tensor_mul(sq, ext, ext)
                    sq3s.append(sq)

            def ssq3_mm(d):
                nc.tensor.matmul(ssq3_ps, oneD_col, sq3s[d],
                                 start=(d == 0), stop=(d == KD - 1))

            for m in range(KD):
                w2_group(m)
                if m >= 2:
                    ssq3_mm(m - 2)
            ssq3_mm(KD - 2)
            ssq3_mm(KD - 1)
            if not last_layer:
                rinv16 = rms_finish(ssq3_ps, T, "rms1")

        # ---------- final norm on extracted columns ----------
        rinvf = rms_finish(ssq3_ps, MSL, "rmsf")
        bcf_ps = ps_mm.tile([128, MSL], F32, tag="mm512")
        nc.tensor.matmul(bcf_ps, ones_row, rinvf, start=True, stop=True)
        for d in range(KD):
            outT = dinvp.tile([128, MSL], F32, tag="outT")
            nc.vector.scalar_tensor_tensor(
                out=outT, in0=exts[d], scalar=lnf_sb[:, d:d + 1], in1=bcf_ps,
                op0=ALU.mult, op1=ALU.mult)
            nc.sync.dma_start(out=out_d[d], in_=outT)

    nc.compile()
    return nc


# =============================================================
# host side
# =============================================================

def _qperm():
    r = np.arange(512)
    h, j2 = r // 64, r % 64
    return np.concatenate([h * 128 + 2 * j2, h * 128 + 2 * j2 + 1])


def prep_inputs(inputs):
    """Build the per-core in_maps (host-side layout/preprocessing only)."""
    ids = np.ascontiguousarray(inputs["input_ids"]).astype(np.int32)
    pos = np.ascontiguousarray(inputs["position_ids"]).astype(np.int32)
    svl = np.ascontiguousarray(inputs["seq_varlen"]).astype(np.int64)
    emb = np.ascontiguousarray(inputs["emb"], dtype=np.float32)
    ln1, ln2, lnf = inputs["ln1"], inputs["ln2"], inputs["lnf"]

    cum = np.cumsum(svl)
    assert cum[-1] == S, "kernel assumes packed tokens fill S exactly"
    seg = np.searchsorted(cum, np.arange(S), side="right")
    for c in range(1, NCORES):
        assert seg[c * T - 1] != seg[c * T], "segment straddles core boundary"
    last_idx = cum - 1
    for c in range(NCORES):
        li = last_idx[c * MSL:(c + 1) * MSL] - c * T
        assert np.array_equal(li, EVLEN - 1 + EVLEN * np.arange(MSL)), \
            "kernel assumes fixed EVLEN segments"

    qperm = _qperm()
    wq = np.empty((L, KD, KD, 128, 128), MMNP)
    wk = np.empty((L, KD, KD, 128, 128), MMNP)
    wv = np.empty((L, KD, 2, 128, 512), MMNP)
    wo = np.empty((L, KD, KD, 128, 128), MMNP)
    w1 = np.empty((L, KF, KD, 128, 128), MMNP)
    w2 = np.empty((L, KD, KF, 128, 128), MMNP)
    for l in range(L):
        g1 = ln1[l][:, None].astype(np.float32)
        g2 = ln2[l][:, None].astype(np.float32)
        Wq_p = (g1 * inputs["Wq"][l])[:, qperm]
        Wk_p = (g1 * inputs["Wk"][l])[:, qperm]
        Wv_p = g1 * inputs["Wv"][l]
        W1_p = g2 * inputs["W1"][l]
        wq[l] = Wq_p.reshape(KD, 128, KD, 128).transpose(2, 0, 1, 3).astype(MMNP)
        wk[l] = Wk_p.reshape(KD, 128, KD, 128).transpose(2, 0, 1, 3).astype(MMNP)
        wv[l] = Wv_p.reshape(KD, 128, 2, 512).transpose(0, 2, 1, 3).astype(MMNP)
        wo[l] = np.asarray(inputs["Wo"][l]).reshape(KD, 128, KD, 128).transpose(2, 0, 1, 3).astype(MMNP)
        w1[l] = W1_p.reshape(KD, 128, KF, 128).transpose(2, 0, 1, 3).astype(MMNP)
        w2[l] = np.asarray(inputs["W2"][l]).reshape(KF, 128, KD, 128).transpose(2, 0, 1, 3).astype(MMNP)

    invf = (1.0 / (ROPE_BASE ** (np.arange(0, DH, 2, dtype=np.float32) / DH)))
    invf2 = np.tile(invf, 2)[:, None].astype(np.float32)
    lnft = np.asarray(lnf, dtype=np.float32).reshape(KD, 128)

    in_maps = []
    for c in range(NCORES):
        sl = slice(c * T, (c + 1) * T)
        seg_c = seg[sl]
        maskT = np.empty((TT, 128, 128), MMNP)
        for t in range(TT):
            sg = seg_c[t * 128:(t + 1) * 128]
            same = (sg[:, None] == sg[None, :])
            kq = np.arange(128)
            maskT[t] = (same & (kq[:, None] <= kq[None, :])).astype(MMNP)
        ang = invf2 * pos[sl].astype(np.float32)[None, :]
        m = {
            "emb": emb,
            "idsc": ids[sl].reshape(TT, 128, 1),
            "maskT": maskT,
            "lnft": lnft,
            "wq": wq, "wk": wk, "wv": wv, "wo": wo, "w1": w1, "w2": w2,
            "costab": np.cos(ang).astype(MMNP),
            "sintab": np.sin(ang).astype(MMNP),
        }
        in_maps.append(m)
    return in_maps


def assemble_output(results):
    """results: list of per-core dicts with 'out' [KD, 128, MSL] -> [8, 16, D]."""
    out = np.empty((NCORES, MSL, D), np.float32)
    for c in range(NCORES):
        a = results[c]["out"]  # [KD, 128, MSL]
        out[c] = a.transpose(2, 0, 1).reshape(MSL, D)
    return out.reshape(NCORES, MSL, D)


_CACHE = {}


def kernel(**inputs) -> np.ndarray:
    from concourse.bass_utils import run_bass_kernel_spmd
    inputs = {k: np.asarray(v) for k, v in inputs.items()}
    if "nc" not in _CACHE:
        _CACHE["nc"] = build_program()
    nc = _CACHE["nc"]
    in_maps = prep_inputs(inputs)
    res = run_bass_kernel_spmd(nc, in_maps, core_ids=list(range(NCORES)))
    return assemble_output(res.results)


# revision 49
# speedup vs baseline: 1.0133x; 1.0133x over previous
"""Trainium2 Bass kernel for nn_EventEncoder (2-layer varlen-packed transformer).

Strategy: sequence-parallel over 8 NeuronCores (512 tokens = 16 whole events per
core, block-diagonal causal attention => no cross-core communication). Weights
replicated, streamed from HBM in bf16. Activations feature-major [D, T].

This revision restructures the schedule around the engine cost model:
 - all hot DVE elementwise ops use bf16 operands in SBUF (4x DVE mode)
 - rmsnorm is deferred: projections read a plain bf16 cast of h; the 1/rms
   factor is folded into the rope tables (q,k) and the V-output scale, so the
   PE never waits on the rmsnorm stat chain in the attention path
 - softmax 1/den via vector.reciprocal; silu via ACT Silu; Sqrt-based rsqrt
   => ~4 ACT table loads per layer instead of ~18
 - rmsnorm/final-norm stats interleave with the residual-add loops
 - embedding gather DMAs are issued first and the transpose/cast pipeline
   follows per 128-token tile

Self-contained: hardcodes all shapes from the problem spec.
"""
import sys
sys.path.insert(0, "/opt/trn_rl_repo")

import numpy as np
import ml_dtypes
from contextlib import ExitStack

import concourse.bass as bass
import concourse.tile as tile
from concourse import bacc, mybir
from concourse.masks import make_identity

# ---- problem constants (hardcoded from spec) ----
S = 4096
NSEG = 128
EVLEN = 32
MSL = 16          # max_seq_len (events per user)
VOCAB = 32002
D = 1024
H = 8
DH = 128
DFF = 4096
L = 2
ROPE_BASE = 10000.0

NCORES = 8
T = S // NCORES       # 512 tokens per core
TT = T // 128         # 4 token tiles
KD = D // 128         # 8 feature tiles
KF = DFF // 128       # 32 ffn tiles
SCALE = 1.0 / float(np.sqrt(DH))

F32 = mybir.dt.float32
BF16 = mybir.dt.bfloat16
I32 = mybir.dt.int32
AF = mybir.ActivationFunctionType
ALU = mybir.AluOpType

MMDT = BF16
MMNP = ml_dtypes.bfloat16


# =============================================================
# device program
# =============================================================

def build_program():
    nc = bacc.Bacc("TRN2", target_bir_lowering=False, debug=False)

    dt_w = MMDT
    # ---- inputs ----
    emb_d = nc.dram_tensor("emb", [VOCAB, D], F32, kind="ExternalInput").ap()
    ids_d = nc.dram_tensor("idsc", [TT, 128, 1], I32, kind="ExternalInput").ap()
    mask_d = nc.dram_tensor("maskT", [TT, 128, 128], MMDT, kind="ExternalInput").ap()
    lnf_d = nc.dram_tensor("lnft", [KD, 128], F32, kind="ExternalInput").ap()
    wq_d = nc.dram_tensor("wq", [L, KD, KD, 128, 128], dt_w, kind="ExternalInput").ap()
    wk_d = nc.dram_tensor("wk", [L, KD, KD, 128, 128], dt_w, kind="ExternalInput").ap()
    wv_d = nc.dram_tensor("wv", [L, KD, 2, 128, 512], dt_w, kind="ExternalInput").ap()
    wo_d = nc.dram_tensor("wo", [L, KD, KD, 128, 128], dt_w, kind="ExternalInput").ap()
    w1_d = nc.dram_tensor("w1", [L, KF, KD, 128, 128], dt_w, kind="ExternalInput").ap()
    w2_d = nc.dram_tensor("w2", [L, KD, KF, 128, 128], dt_w, kind="ExternalInput").ap()
    cs_d = nc.dram_tensor("costab", [128, T], MMDT, kind="ExternalInput").ap()
    sn_d = nc.dram_tensor("sintab", [128, T], MMDT, kind="ExternalInput").ap()

    out_d = nc.dram_tensor("out", [KD, 128, MSL], F32, kind="ExternalOutput").ap()

    with tile.TileContext(nc) as tc, ExitStack() as ctx:
        persist = ctx.enter_context(tc.tile_pool(name="persist", bufs=1))
        acts = ctx.enter_context(tc.tile_pool(name="acts", bufs=1))
        wpool = ctx.enter_context(tc.tile_pool(name="wpool", bufs=6))
        w2pool = ctx.enter_context(tc.tile_pool(name="w2pool", bufs=2))
        tmp = ctx.enter_context(tc.tile_pool(name="tmp", bufs=4))
        eodp = ctx.enter_context(tc.tile_pool(name="eodp", bufs=4))
        epool = ctx.enter_context(tc.tile_pool(name="epool", bufs=9))
        dinvp = ctx.enter_context(tc.tile_pool(name="dinvp", bufs=4))
        dbcp = ctx.enter_context(tc.tile_pool(name="dbcp", bufs=3))
        wqpre = ctx.enter_context(tc.tile_pool(name="wqpre", bufs=2))
        gpool = ctx.enter_context(tc.tile_pool(name="gpool", bufs=2))
        rowp = ctx.enter_context(tc.tile_pool(name="rowp", bufs=1))
        ps_mm = ctx.enter_context(tc.tile_pool(name="ps_mm", bufs=4, space="PSUM"))
        ps_att = ctx.enter_context(tc.tile_pool(name="ps_att", bufs=2, space="PSUM"))
        ps_row = ps_att  # row-psums ([1, n]) borrow the att_o slots

        # ---------- embedding gather issued FIRST ----------
        g_tiles = []
        for t in range(TT):
            ids_sb = dinvp.tile([128, 1], I32, tag="ids_sb")
            nc.sync.dma_start(out=ids_sb, in_=ids_d[t])
            g = gpool.tile([128, D], F32, tag="g")
            nc.gpsimd.indirect_dma_start(
                out=g[:], out_offset=None, in_=emb_d[:],
                in_offset=bass.IndirectOffsetOnAxis(ap=ids_sb[:, 0:1], axis=0),
            )
            g_tiles.append(g)

        # ---------- persistent tiles / other DMAs (overlap the gather) ----------
        hT = persist.tile([128, KD, T], F32, tag="hT")
        h16 = persist.tile([128, KD, T], MMDT, tag="h16")
        ident = persist.tile([128, 128], F32, tag="ident")
        make_identity(nc, ident)
        oneD_col = persist.tile([128, 1], MMDT, tag="oneD_col")  # 1/D for ssq
        nc.vector.memset(oneD_col, 1.0 / D)
        ones_col = persist.tile([128, 1], MMDT, tag="ones_col")  # 1.0 for den
        nc.vector.memset(ones_col, 1.0)
        ones_row = persist.tile([1, 128], MMDT, tag="ones_row")  # K=1 -> bcast
        nc.vector.memset(ones_row, 1.0)
        ones11 = persist.tile([1, 1], MMDT, tag="ones11")
        nc.vector.memset(ones11, 1.0)
        warm_in = persist.tile([1, 1], F32, tag="warm_in")
        nc.vector.memset(warm_in, 1.0)
        warm_out = persist.tile([1, 1], F32, tag="warm_out")

        def act_warm(func, anchor):
            # dummy activation anchored on `anchor` (a [1,1] AP): pins the ACT
            # table load into an idle window instead of the critical chain
            nc.scalar.activation(out=warm_out, in_=anchor, func=func)
        mask_sb = persist.tile([128, TT, 128], MMDT, tag="mask_sb")
        nc.sync.dma_start(out=mask_sb, in_=mask_d.transpose([1, 0, 2]))
        mask_flat = mask_sb.rearrange("p t q -> p (t q)")
        lnf_sb = persist.tile([128, KD], F32, tag="lnf_sb")
        nc.sync.dma_start(out=lnf_sb, in_=lnf_d.transpose([1, 0]))
        cos16 = persist.tile([128, T], MMDT, tag="cos16")
        sin16 = persist.tile([128, T], MMDT, tag="sin16")
        nc.sync.dma_start(out=cos16, in_=cs_d)
        nc.sync.dma_start(out=sin16, in_=sn_d)

        def prefetch_wq(l_):
            wg_e = wqpre.tile([128, KD, 128], dt_w, tag="wqpre")
            nc.sync.dma_start(out=wg_e, in_=wq_d[l_, 0].transpose([1, 0, 2]))
            wg_o = wqpre.tile([128, KD, 128], dt_w, tag="wqpre")
            nc.sync.dma_start(out=wg_o, in_=wq_d[l_, 4].transpose([1, 0, 2]))
            return wg_e, wg_o

        # wv for layer 0 streams during the gather (fine-grained for DMA-queue
        # parallelism; layer-0 V projections need it ~15us in)
        wv_sb = acts.tile([128, KD, 2, 512], dt_w, tag="wv_sb")
        for nh_ in range(2):
            for q in range(4):
                hs = slice(q * 2, q * 2 + 2)
                nc.sync.dma_start(out=wv_sb[:, hs, nh_, :],
                                  in_=wv_d[0, hs, nh_].transpose([1, 0, 2]))
        v_sb = acts.tile([128, TT, 2, 512], MMDT, tag="v_sb")
        pre_q = prefetch_wq(0)

        # ---------- gather -> transpose -> hT/h16; layer-0 V projections
        # interleave per token tile to fill the PE during the gather tail
        # (the rinv scale is applied to v_sb in place once stats are ready) ----
        for t in range(TT):
            for d in range(KD):
                tp_ps = ps_mm.tile([128, 128], F32, tag="mm512")
                nc.tensor.transpose(out=tp_ps, in_=g_tiles[t][:, d * 128:(d + 1) * 128],
                                    identity=ident)
                cs = slice(t * 128, (t + 1) * 128)
                nc.vector.tensor_copy(out=hT[:, d, cs], in_=tp_ps)
                nc.scalar.copy(out=h16[:, d, cs], in_=tp_ps)
                if t == 0 and d == 0:
                    act_warm(AF.Sqrt, h16[0:1, 0, 0:1])
            for nh_v in range(2):
                v_ps = ps_mm.tile([128, 512], F32, tag="mm512")
                for kt in range(KD):
                    nc.tensor.matmul(
                        v_ps,
                        h16[:, kt, t * 128:(t + 1) * 128],
                        wv_sb[:, kt, nh_v, :],
                        start=(kt == 0), stop=(kt == KD - 1))
                nc.scalar.copy(out=v_sb[:, t, nh_v, :], in_=v_ps)

        # ---------- helpers ----------
        def rms_stats(sq_src, n_free, tag):
            """sq_src: list of KD bf16 sq tiles [128, n_free] (already squared).
            Returns rinv16 [1, n_free] bf16 = rsqrt(mean + eps)."""
            ssq_ps = ps_row.tile([1, n_free], F32, tag="att_o")
            for d in range(KD):
                nc.tensor.matmul(ssq_ps, oneD_col, sq_src[d],
                                 start=(d == 0), stop=(d == KD - 1))
            return rms_finish(ssq_ps, n_free, tag, eps=True)

        def rms_finish(ssq_ps, n_free, tag, eps=False):
            # eps=1e-6 matters only at the embedding scale (layer-0 rms1);
            # everywhere else mean-sq >= ~0.2 and the add is skipped.
            src_ap = ssq_ps
            if eps:
                m_eps = rowp.tile([1, n_free], F32, tag="me")
                nc.vector.tensor_scalar_add(m_eps, ssq_ps, 1e-6)
                src_ap = m_eps
            rec = rowp.tile([1, n_free], F32, tag="rc")
            nc.vector.reciprocal_approx_fast(out=rec, in_=src_ap)
            rinv16 = rowp.tile([1, n_free], MMDT, tag="ri")
            nc.scalar.activation(out=rinv16, in_=rec, func=AF.Sqrt)
            return rinv16

        def bcast16(rinv16, n_free, tag):
            """broadcast [1, n] bf16 row to [128, n] bf16 SBUF tile."""
            bc_ps = ps_mm.tile([128, n_free], F32, tag="mm512")
            nc.tensor.matmul(bc_ps, ones_row, rinv16, start=True, stop=True)
            bc = acts.tile([128, n_free], MMDT, tag=f"bc_{tag}")
            nc.scalar.copy(out=bc, in_=bc_ps)
            return bc

        # ---------- layer-0 rmsnorm stats (pipelined after casts) ----------
        sqs = []
        for d in range(KD):
            sq = tmp.tile([128, T], MMDT, tag="sq")
            nc.vector.tensor_mul(sq, h16[:, d, :], h16[:, d, :])
            sqs.append(sq)
        rinv16 = rms_stats(sqs, T, "rms1")

        # ---------- layers ----------
        for l in range(L):
            # ---- Q, K projections (read h16 directly) + rope (bf16, 4x DVE) ----
            qrot = acts.tile([128, KD, T], MMDT, tag="qrot")
            krot = acts.tile([128, KD, T], MMDT, tag="krot")
            cosL = acts.tile([128, T], MMDT, tag="cosL")
            sinL = acts.tile([128, T], MMDT, tag="sinL")
            rinv_col = persist.tile([128, TT], F32, tag=f"rcol{l}")

            def qk_group(w_d_, pair, pre=None):
                if pre is not None:
                    wg_e, wg_o = pre
                else:
                    wg_e = wpool.tile([128, KD, 128], dt_w, tag="wtile")
                    nc.sync.dma_start(out=wg_e, in_=w_d_[l, pair].transpose([1, 0, 2]))
                    wg_o = wpool.tile([128, KD, 128], dt_w, tag="wtile")
                    nc.sync.dma_start(out=wg_o, in_=w_d_[l, pair + 4].transpose([1, 0, 2]))
                ev_ps = ps_mm.tile([128, T], F32, tag="mm512")
                od_ps = ps_mm.tile([128, T], F32, tag="mm512")
                for kt in range(KD):
                    nc.tensor.matmul(ev_ps, wg_e[:, kt, :], h16[:, kt, :],
                                     start=(kt == 0), stop=(kt == KD - 1))
                for kt in range(KD):
                    nc.tensor.matmul(od_ps, wg_o[:, kt, :], h16[:, kt, :],
                                     start=(kt == 0), stop=(kt == KD - 1))
                return ev_ps, od_ps

            def rope_apply(rot, pair, ev_ps, od_ps):
                ev16 = eodp.tile([128, T], MMDT, tag="eod")
                od16 = eodp.tile([128, T], MMDT, tag="eod")
                nc.scalar.copy(out=ev16, in_=ev_ps)
                nc.scalar.copy(out=od16, in_=od_ps)
                t1 = tmp.tile([128, T], MMDT, tag="rtmp")
                t2 = tmp.tile([128, T], MMDT, tag="rtmp")
                nc.vector.tensor_mul(t1, ev16, cosL)
                nc.vector.tensor_mul(t2, od16, sinL)
                nc.vector.tensor_sub(rot[:, pair, :], t1, t2)
                t3 = tmp.tile([128, T], MMDT, tag="rtmp")
                t4 = tmp.tile([128, T], MMDT, tag="rtmp")
                nc.vector.tensor_mul(t3, ev16, sinL)
                nc.vector.tensor_mul(t4, od16, cosL)
                nc.vector.tensor_add(rot[:, pair + 4, :], t3, t4)

            # first q group goes ahead of the rinv-dependent table prep so the
            # PE never idles waiting on the rmsnorm chain
            ev0, od0 = qk_group(wq_d, 0, pre=pre_q)
            act_warm(AF.Exp, rinv16[0:1, 0:1])
            bc1 = bcast16(rinv16, T, "r1")
            nc.vector.tensor_mul(cosL, cos16, bc1)
            nc.vector.tensor_mul(sinL, sin16, bc1)
            for t in range(TT):
                rc_ps = ps_row.tile([128, 1], F32, tag="att_o")
                nc.tensor.matmul(rc_ps, rinv16[:, t * 128:(t + 1) * 128], ones11,
                                 start=True, stop=True)
                nc.vector.tensor_copy(out=rinv_col[:, t:t + 1], in_=rc_ps)
            rope_apply(qrot, 0, ev0, od0)
            for pair in range(4):
                for (w_d_, rot) in ((wq_d, qrot), (wk_d, krot)):
                    if pair == 0 and rot is qrot:
                        continue
                    ev_ps, od_ps = qk_group(w_d_, pair)
                    rope_apply(rot, pair, ev_ps, od_ps)

            # ---- V projections (read h16; rinv applied on the psum->sbuf copy).
            # Layer 0's V matmuls already ran in the prologue: only the rinv
            # scale fixup remains (in place).
            if l == 0:
                for t_v in range(TT):
                    for nh_v in range(2):
                        nc.scalar.activation(
                            out=v_sb[:, t_v, nh_v, :], in_=v_sb[:, t_v, nh_v, :],
                            func=AF.Copy, scale=rinv_col[:, t_v:t_v + 1])
            else:
                for t_v in range(TT):
                    for nh_v in range(2):
                        v_ps = ps_mm.tile([128, 512], F32, tag="mm512")
                        for kt in range(KD):
                            nc.tensor.matmul(
                                v_ps,
                                h16[:, kt, t_v * 128:(t_v + 1) * 128],
                                wv_sb[:, kt, nh_v, :],
                                start=(kt == 0), stop=(kt == KD - 1))
                        nc.scalar.activation(out=v_sb[:, t_v, nh_v, :], in_=v_ps,
                                             func=AF.Copy, scale=rinv_col[:, t_v:t_v + 1])

            # ---- attention (software-pipelined: den lags scores by 1 head,
            # dbc + attn@V lag by 2, so the PE never waits on the ACT/DVE
            # softmax chain) ----
            oT = acts.tile([128, KD, T], MMDT, tag="oT")
            e_tiles = {}
            den_tiles = {}
            dbc_tiles = {}

            def scores(h):
                me, mo, off = h // 2, 4 + h // 2, (h % 2) * 64
                s_ps = ps_att.tile([128, T], F32, tag="att_s")
                for t in range(TT):
                    ts_ = slice(t * 128, (t + 1) * 128)
                    nc.tensor.matmul(s_ps[:, ts_], krot[off:off + 64, me, ts_],
                                     qrot[off:off + 64, me, ts_],
                                     start=True, stop=False)
                    nc.tensor.matmul(s_ps[:, ts_], krot[off:off + 64, mo, ts_],
                                     qrot[off:off + 64, mo, ts_],
                                     start=False, stop=True)
                ef16 = tmp.tile([128, T], MMDT, tag="ef16")
                nc.scalar.activation(out=ef16, in_=s_ps, func=AF.Exp, scale=float(SCALE))
                em = epool.tile([128, T], MMDT, tag="e_mm")
                nc.vector.tensor_mul(em, ef16, mask_flat)
                e_tiles[h] = em

            def den(h):
                den_ps = ps_row.tile([1, T], F32, tag="att_o")
                nc.tensor.matmul(den_ps, ones_col, e_tiles[h], start=True, stop=True)
                dinv_f = dinvp.tile([1, T], F32, tag="dinv_f")
                nc.vector.reciprocal_approx_fast(out=dinv_f, in_=den_ps)
                dinv16 = dinvp.tile([1, T], MMDT, tag="dinv16")
                nc.vector.tensor_copy(out=dinv16, in_=dinv_f)
                den_tiles[h] = dinv16

            def attn_v(h):
                dbc_ps = ps_mm.tile([128, T], F32, tag="mm512")
                nc.tensor.matmul(dbc_ps, ones_row, den_tiles[h], start=True, stop=True)
                dbc16 = dbcp.tile([128, T], MMDT, tag="dbc16")
                nc.scalar.copy(out=dbc16, in_=dbc_ps)
                o_ps = ps_att.tile([128, T], F32, tag="att_o")
                for t in range(TT):
                    ts_ = slice(t * 128, (t + 1) * 128)
                    nc.tensor.matmul(
                        o_ps[:, ts_],
                        v_sb[:, t, h // 4, (h % 4) * 128:(h % 4 + 1) * 128],
                        e_tiles[h][:, ts_], start=True, stop=True)
                nc.vector.tensor_mul(oT[:, h, :], o_ps, dbc16)

            for h in range(H):
                scores(h)
                if h >= 2:
                    den(h - 2)
                if h >= 3:
                    attn_v(h - 3)
            den(H - 2)
            den(H - 1)
            attn_v(H - 3)
            attn_v(H - 2)
            attn_v(H - 1)
            act_warm(AF.Sqrt, e_tiles[H - 1][0:1, 0:1])

            # ---- Wo + residual; rmsnorm2 stats interleaved (2-group lag) ----
            sq2s = []
            ssq2_ps = ps_row.tile([1, T], F32, tag="att_o")

            def wo_group(m):
                wg = wpool.tile([128, KD, 128], dt_w, tag="wtile")
                nc.sync.dma_start(out=wg, in_=wo_d[l, m].transpose([1, 0, 2]))
                wo_ps = ps_mm.tile([128, T], F32, tag="mm512")
                for kt in range(KD):
                    nc.tensor.matmul(wo_ps, wg[:, kt, :], oT[:, kt, :],
                                     start=(kt == 0), stop=(kt == KD - 1))
                nc.vector.tensor_add(hT[:, m, :], hT[:, m, :], wo_ps)
                if m < KD - 1:  # last cast deferred past the rms Sqrt (ACT order)
                    nc.scalar.copy(out=h16[:, m, :], in_=hT[:, m, :])
                sq = tmp.tile([128, T], MMDT, tag="sq")
                nc.vector.tensor_mul(sq, hT[:, m, :], hT[:, m, :])
                sq2s.append(sq)

            def ssq2_mm(d):
                nc.tensor.matmul(ssq2_ps, oneD_col, sq2s[d],
                                 start=(d == 0), stop=(d == KD - 1))

            for m in range(KD):
                wo_group(m)
                if m >= 2:
                    ssq2_mm(m - 2)
            ssq2_mm(KD - 2)
            ssq2_mm(KD - 1)
            rinv2 = rms_finish(ssq2_ps, T, "rms2")
            bc2 = bcast16(rinv2, T, "r2")
            act_warm(AF.Silu, bc2[0:1, 0:1])
            nc.scalar.copy(out=h16[:, KD - 1, :], in_=hT[:, KD - 1, :])

            # ---- MLP ----
            x2T = acts.tile([128, KD, T], MMDT, tag="x2T")
            for d in range(KD):
                nc.vector.tensor_mul(x2T[:, d, :], h16[:, d, :], bc2)
            y1 = acts.tile([128, KF, 512], MMDT, tag="y1")
            for j in range(KF):
                wg = wpool.tile([128, KD, 128], dt_w, tag="wtile")
                nc.sync.dma_start(out=wg, in_=w1_d[l, j].transpose([1, 0, 2]))
                y1_ps = ps_mm.tile([128, T], F32, tag="mm512")
                for kt in range(KD):
                    nc.tensor.matmul(y1_ps, wg[:, kt, :], x2T[:, kt, :],
                                     start=(kt == 0), stop=(kt == KD - 1))
                nc.scalar.activation(out=y1[:, j, :], in_=y1_ps, func=AF.Silu)
            act_warm(AF.Sqrt, y1[0:1, KF - 1, 0:1])

            last_layer = (l == L - 1)
            # prefetch next layer's wv + first q-pair weights ahead of the W2
            # DMA storm so the next layer's QK phase starts without a DMA wait
            if not last_layer:
                for nh_ in range(2):
                    nc.sync.dma_start(out=wv_sb[:, :, nh_, :],
                                      in_=wv_d[l + 1, :, nh_].transpose([1, 0, 2]))
                pre_q = prefetch_wq(l + 1)
            sq3s = []
            ssq3_ps = ps_row.tile([1, T if not last_layer else MSL], F32, tag="att_o")
            exts = []

            def w2_group(m):
                wg2 = w2pool.tile([128, KF, 128], dt_w, tag="w2tile")
                for half in range(2):
                    hs = slice(half * (KF // 2), (half + 1) * (KF // 2))
                    nc.sync.dma_start(out=wg2[:, hs, :],
                                      in_=w2_d[l, m, hs].transpose([1, 0, 2]))
                y2_ps = ps_mm.tile([128, T], F32, tag="mm512")
                for j in range(KF):
                    nc.tensor.matmul(y2_ps, wg2[:, j, :], y1[:, j, :],
                                     start=(j == 0), stop=(j == KF - 1))
                nc.vector.tensor_add(hT[:, m, :], hT[:, m, :], y2_ps)
                if not last_layer:
                    nc.scalar.copy(out=h16[:, m, :], in_=hT[:, m, :])
                    sq = tmp.tile([128, T], MMDT, tag="sq")
                    nc.vector.tensor_mul(sq, hT[:, m, :], hT[:, m, :])
                    sq3s.append(sq)
                else:
                    # extract last-token columns, square them (final norm stats)
                    ext = persist.tile([128, MSL], F32, tag=f"ext{m}")
                    src = hT[:, m, :].rearrange("p (e w) -> p e w", w=EVLEN)[:, :, EVLEN - 1]
                    nc.vector.tensor_copy(out=ext, in_=src)
                    exts.append(ext)
                    sq = tmp.tile([128, MSL], MMDT, tag="sqe")
                    nc.vector.tensor_mul(sq, ext, ext)
                    sq3s.append(sq)

            def ssq3_mm(d):
                nc.tensor.matmul(ssq3_ps, oneD_col, sq3s[d],
                                 start=(d == 0), stop=(d == KD - 1))

            for m in range(KD):
                w2_group(m)
                if m >= 2:
                    ssq3_mm(m - 2)
            ssq3_mm(KD - 2)
            ssq3_mm(KD - 1)
            if not last_layer:
                rinv16 = rms_finish(ssq3_ps, T, "rms1")

        # ---------- final norm on extracted columns ----------
        rinvf = rms_finish(ssq3_ps, MSL, "rmsf")
        bcf_ps = ps_mm.tile([128, MSL], F32, tag="mm512")
        nc.tensor.matmul(bcf_ps, ones_row, rinvf, start=True, stop=True)
        for d in range(KD):
            outT = dinvp.tile([128, MSL], F32, tag="outT")
            nc.vector.scalar_tensor_tensor(
                out=outT, in0=exts[d], scalar=lnf_sb[:, d:d + 1], in1=bcf_ps,
                op0=ALU.mult, op1=ALU.mult)
            nc.sync.dma_start(out=out_d[d], in_=outT)

    nc.compile()
    return nc


# =============================================================
# host side
# =============================================================

def _qperm():
    r = np.arange(512)
    h, j2 = r // 64, r % 64
    return np.concatenate([h * 128 + 2 * j2, h * 128 + 2 * j2 + 1])


def prep_inputs(inputs):
    """Build the per-core in_maps (host-side layout/preprocessing only)."""
    ids = np.ascontiguousarray(inputs["input_ids"]).astype(np.int32)
    pos = np.ascontiguousarray(inputs["position_ids"]).astype(np.int32)
    svl = np.ascontiguousarray(inputs["seq_varlen"]).astype(np.int64)
    emb = np.ascontiguousarray(inputs["emb"], dtype=np.float32)
    ln1, ln2, lnf = inputs["ln1"], inputs["ln2"], inputs["lnf"]

    cum = np.cumsum(svl)
    assert cum[-1] == S, "kernel assumes packed tokens fill S exactly"
    seg = np.searchsorted(cum, np.arange(S), side="right")
    for c in range(1, NCORES):
        assert seg[c * T - 1] != seg[c * T], "segment straddles core boundary"
    last_idx = cum - 1
    for c in range(NCORES):
        li = last_idx[c * MSL:(c + 1) * MSL] - c * T
        assert np.array_equal(li, EVLEN - 1 + EVLEN * np.arange(MSL)), \
            "kernel assumes fixed EVLEN segments"

    qperm = _qperm()
    wq = np.empty((L, KD, KD, 128, 128), MMNP)
    wk = np.empty((L, KD, KD, 128, 128), MMNP)
    wv = np.empty((L, KD, 2, 128, 512), MMNP)
    wo = np.empty((L, KD, KD, 128, 128), MMNP)
    w1 = np.empty((L, KF, KD, 128, 128), MMNP)
    w2 = np.empty((L, KD, KF, 128, 128), MMNP)
    for l in range(L):
        g1 = ln1[l][:, None].astype(np.float32)
        g2 = ln2[l][:, None].astype(np.float32)
        Wq_p = (g1 * inputs["Wq"][l])[:, qperm]
        Wk_p = (g1 * inputs["Wk"][l])[:, qperm]
        Wv_p = g1 * inputs["Wv"][l]
        W1_p = g2 * inputs["W1"][l]
        wq[l] = Wq_p.reshape(KD, 128, KD, 128).transpose(2, 0, 1, 3).astype(MMNP)
        wk[l] = Wk_p.reshape(KD, 128, KD, 128).transpose(2, 0, 1, 3).astype(MMNP)
        wv[l] = Wv_p.reshape(KD, 128, 2, 512).transpose(0, 2, 1, 3).astype(MMNP)
        wo[l] = np.asarray(inputs["Wo"][l]).reshape(KD, 128, KD, 128).transpose(2, 0, 1, 3).astype(MMNP)
        w1[l] = W1_p.reshape(KD, 128, KF, 128).transpose(2, 0, 1, 3).astype(MMNP)
        w2[l] = np.asarray(inputs["W2"][l]).reshape(KF, 128, KD, 128).transpose(2, 0, 1, 3).astype(MMNP)

    invf = (1.0 / (ROPE_BASE ** (np.arange(0, DH, 2, dtype=np.float32) / DH)))
    invf2 = np.tile(invf, 2)[:, None].astype(np.float32)
    lnft = np.asarray(lnf, dtype=np.float32).reshape(KD, 128)

    in_maps = []
    for c in range(NCORES):
        sl = slice(c * T, (c + 1) * T)
        seg_c = seg[sl]
        maskT = np.empty((TT, 128, 128), MMNP)
        for t in range(TT):
            sg = seg_c[t * 128:(t + 1) * 128]
            same = (sg[:, None] == sg[None, :])
            kq = np.arange(128)
            maskT[t] = (same & (kq[:, None] <= kq[None, :])).astype(MMNP)
        ang = invf2 * pos[sl].astype(np.float32)[None, :]
        m = {
            "emb": emb,
            "idsc": ids[sl].reshape(TT, 128, 1),
            "maskT": maskT,
            "lnft": lnft,
            "wq": wq, "wk": wk, "wv": wv, "wo": wo, "w1": w1, "w2": w2,
            "costab": np.cos(ang).astype(MMNP),
            "sintab": np.sin(ang).astype(MMNP),
        }
        in_maps.append(m)
    return in_maps


def assemble_output(results):
    """results: list of per-core dicts with 'out' [KD, 128, MSL] -> [8, 16, D]."""
    out = np.empty((NCORES, MSL, D), np.float32)
    for c in range(NCORES):
        a = results[c]["out"]  # [KD, 128, MSL]
        out[c] = a.transpose(2, 0, 1).reshape(MSL, D)
    return out.reshape(NCORES, MSL, D)


_CACHE = {}


def kernel(**inputs) -> np.ndarray:
    from concourse.bass_utils import run_bass_kernel_spmd
    inputs = {k: np.asarray(v) for k, v in inputs.items()}
    if "nc" not in _CACHE:
        _CACHE["nc"] = build_program()
    nc = _CACHE["nc"]
    in_maps = prep_inputs(inputs)
    res = run_bass_kernel_spmd(nc, in_maps, core_ids=list(range(NCORES)))
    return assemble_output(res.results)


# revision 50
# speedup vs baseline: 1.0261x; 1.0127x over previous
"""Trainium2 Bass kernel for nn_EventEncoder (2-layer varlen-packed transformer).

Strategy: sequence-parallel over 8 NeuronCores (512 tokens = 16 whole events per
core, block-diagonal causal attention => no cross-core communication). Weights
replicated, streamed from HBM in bf16. Activations feature-major [D, T].

This revision restructures the schedule around the engine cost model:
 - all hot DVE elementwise ops use bf16 operands in SBUF (4x DVE mode)
 - rmsnorm is deferred: projections read a plain bf16 cast of h; the 1/rms
   factor is folded into the rope tables (q,k) and the V-output scale, so the
   PE never waits on the rmsnorm stat chain in the attention path
 - softmax 1/den via vector.reciprocal; silu via ACT Silu; Sqrt-based rsqrt
   => ~4 ACT table loads per layer instead of ~18
 - rmsnorm/final-norm stats interleave with the residual-add loops
 - embedding gather DMAs are issued first and the transpose/cast pipeline
   follows per 128-token tile

Self-contained: hardcodes all shapes from the problem spec.
"""
import sys
sys.path.insert(0, "/opt/trn_rl_repo")

import numpy as np
import ml_dtypes
from contextlib import ExitStack

import concourse.bass as bass
import concourse.tile as tile
from concourse import bacc, mybir
from concourse.masks import make_identity

# ---- problem constants (hardcoded from spec) ----
S = 4096
NSEG = 128
EVLEN = 32
MSL = 16          # max_seq_len (events per user)
VOCAB = 32002
D = 1024
H = 8
DH = 128
DFF = 4096
L = 2
ROPE_BASE = 10000.0

NCORES = 8
T = S // NCORES       # 512 tokens per core
TT = T // 128         # 4 token tiles
KD = D // 128         # 8 feature tiles
KF = DFF // 128       # 32 ffn tiles
SCALE = 1.0 / float(np.sqrt(DH))

F32 = mybir.dt.float32
BF16 = mybir.dt.bfloat16
I32 = mybir.dt.int32
AF = mybir.ActivationFunctionType
ALU = mybir.AluOpType

MMDT = BF16
MMNP = ml_dtypes.bfloat16


# =============================================================
# device program
# =============================================================

def build_program():
    nc = bacc.Bacc("TRN2", target_bir_lowering=False, debug=False)

    dt_w = MMDT
    # ---- inputs ----
    emb_d = nc.dram_tensor("emb", [VOCAB, D], F32, kind="ExternalInput").ap()
    ids_d = nc.dram_tensor("idsc", [TT, 128, 1], I32, kind="ExternalInput").ap()
    mask_d = nc.dram_tensor("maskT", [TT, 128, 128], MMDT, kind="ExternalInput").ap()
    lnf_d = nc.dram_tensor("lnft", [KD, 128], F32, kind="ExternalInput").ap()
    wq_d = nc.dram_tensor("wq", [L, KD, KD, 128, 128], dt_w, kind="ExternalInput").ap()
    wk_d = nc.dram_tensor("wk", [L, KD, KD, 128, 128], dt_w, kind="ExternalInput").ap()
    wv_d = nc.dram_tensor("wv", [L, KD, 2, 128, 512], dt_w, kind="ExternalInput").ap()
    wo_d = nc.dram_tensor("wo", [L, KD, KD, 128, 128], dt_w, kind="ExternalInput").ap()
    w1_d = nc.dram_tensor("w1", [L, KF, KD, 128, 128], dt_w, kind="ExternalInput").ap()
    w2_d = nc.dram_tensor("w2", [L, KD, KF, 128, 128], dt_w, kind="ExternalInput").ap()
    cs_d = nc.dram_tensor("costab", [128, T], MMDT, kind="ExternalInput").ap()
    sn_d = nc.dram_tensor("sintab", [128, T], MMDT, kind="ExternalInput").ap()

    out_d = nc.dram_tensor("out", [KD, 128, MSL], F32, kind="ExternalOutput").ap()

    with tile.TileContext(nc) as tc, ExitStack() as ctx:
        persist = ctx.enter_context(tc.tile_pool(name="persist", bufs=1))
        acts = ctx.enter_context(tc.tile_pool(name="acts", bufs=1))
        wpool = ctx.enter_context(tc.tile_pool(name="wpool", bufs=6))
        w2pool = ctx.enter_context(tc.tile_pool(name="w2pool", bufs=2))
        tmp = ctx.enter_context(tc.tile_pool(name="tmp", bufs=4))
        eodp = ctx.enter_context(tc.tile_pool(name="eodp", bufs=4))
        epool = ctx.enter_context(tc.tile_pool(name="epool", bufs=9))
        dinvp = ctx.enter_context(tc.tile_pool(name="dinvp", bufs=4))
        dbcp = ctx.enter_context(tc.tile_pool(name="dbcp", bufs=3))
        wqpre = ctx.enter_context(tc.tile_pool(name="wqpre", bufs=2))
        gpool = ctx.enter_context(tc.tile_pool(name="gpool", bufs=2))
        rowp = ctx.enter_context(tc.tile_pool(name="rowp", bufs=1))
        ps_mm = ctx.enter_context(tc.tile_pool(name="ps_mm", bufs=4, space="PSUM"))
        ps_att = ctx.enter_context(tc.tile_pool(name="ps_att", bufs=2, space="PSUM"))
        ps_row = ps_att  # row-psums ([1, n]) borrow the att_o slots

        # ---------- embedding gather issued FIRST ----------
        g_tiles = []
        for t in range(TT):
            ids_sb = dinvp.tile([128, 1], I32, tag="ids_sb")
            nc.sync.dma_start(out=ids_sb, in_=ids_d[t])
            g = gpool.tile([128, D], F32, tag="g")
            nc.gpsimd.indirect_dma_start(
                out=g[:], out_offset=None, in_=emb_d[:],
                in_offset=bass.IndirectOffsetOnAxis(ap=ids_sb[:, 0:1], axis=0),
            )
            g_tiles.append(g)

        # ---------- persistent tiles / other DMAs (overlap the gather) ----------
        hT = persist.tile([128, KD, T], F32, tag="hT")
        h16 = persist.tile([128, KD, T], MMDT, tag="h16")
        ident = persist.tile([128, 128], F32, tag="ident")
        make_identity(nc, ident)
        oneD_col = persist.tile([128, 1], MMDT, tag="oneD_col")  # 1/D for ssq
        nc.vector.memset(oneD_col, 1.0 / D)
        ones_col = persist.tile([128, 1], MMDT, tag="ones_col")  # 1.0 for den
        nc.vector.memset(ones_col, 1.0)
        ones_row = persist.tile([1, 128], MMDT, tag="ones_row")  # K=1 -> bcast
        nc.vector.memset(ones_row, 1.0)
        ones11 = persist.tile([1, 1], MMDT, tag="ones11")
        nc.vector.memset(ones11, 1.0)
        warm_in = persist.tile([1, 1], F32, tag="warm_in")
        nc.vector.memset(warm_in, 1.0)
        warm_out = persist.tile([1, 1], F32, tag="warm_out")

        def act_warm(func, anchor):
            # dummy activation anchored on `anchor` (a [1,1] AP): pins the ACT
            # table load into an idle window instead of the critical chain
            nc.scalar.activation(out=warm_out, in_=anchor, func=func)
        mask_sb = persist.tile([128, TT, 128], MMDT, tag="mask_sb")
        nc.sync.dma_start(out=mask_sb, in_=mask_d.transpose([1, 0, 2]))
        mask_flat = mask_sb.rearrange("p t q -> p (t q)")
        lnf_sb = persist.tile([128, KD], F32, tag="lnf_sb")
        nc.sync.dma_start(out=lnf_sb, in_=lnf_d.transpose([1, 0]))
        cos16 = persist.tile([128, T], MMDT, tag="cos16")
        sin16 = persist.tile([128, T], MMDT, tag="sin16")
        nc.sync.dma_start(out=cos16, in_=cs_d)
        nc.sync.dma_start(out=sin16, in_=sn_d)

        def prefetch_wq(l_):
            wg_e = wqpre.tile([128, KD, 128], dt_w, tag="wqpre")
            nc.sync.dma_start(out=wg_e, in_=wq_d[l_, 0].transpose([1, 0, 2]))
            wg_o = wqpre.tile([128, KD, 128], dt_w, tag="wqpre")
            nc.sync.dma_start(out=wg_o, in_=wq_d[l_, 4].transpose([1, 0, 2]))
            return wg_e, wg_o

        # wv for layer 0 streams during the gather (fine-grained for DMA-queue
        # parallelism; layer-0 V projections need it ~15us in)
        wv_sb = acts.tile([128, KD, 2, 512], dt_w, tag="wv_sb")
        for nh_ in range(2):
            for q in range(4):
                hs = slice(q * 2, q * 2 + 2)
                nc.sync.dma_start(out=wv_sb[:, hs, nh_, :],
                                  in_=wv_d[0, hs, nh_].transpose([1, 0, 2]))
        v_sb = acts.tile([128, TT, 2, 512], MMDT, tag="v_sb")
        pre_q = prefetch_wq(0)

        # ---------- gather -> transpose -> hT/h16; layer-0 V projections
        # interleave per token tile to fill the PE during the gather tail
        # (the rinv scale is applied to v_sb in place once stats are ready) ----
        for t in range(TT):
            for d in range(KD):
                tp_ps = ps_mm.tile([128, 128], F32, tag="mm512")
                nc.tensor.transpose(out=tp_ps, in_=g_tiles[t][:, d * 128:(d + 1) * 128],
                                    identity=ident)
                cs = slice(t * 128, (t + 1) * 128)
                nc.vector.tensor_copy(out=hT[:, d, cs], in_=tp_ps)
                nc.scalar.copy(out=h16[:, d, cs], in_=tp_ps)
                if t == 0 and d == 0:
                    act_warm(AF.Abs_reciprocal_sqrt, h16[0:1, 0, 0:1])
            for nh_v in range(2):
                v_ps = ps_mm.tile([128, 512], F32, tag="mm512")
                for kt in range(KD):
                    nc.tensor.matmul(
                        v_ps,
                        h16[:, kt, t * 128:(t + 1) * 128],
                        wv_sb[:, kt, nh_v, :],
                        start=(kt == 0), stop=(kt == KD - 1))
                nc.scalar.copy(out=v_sb[:, t, nh_v, :], in_=v_ps)

        # ---------- helpers ----------
        def rms_stats(sq_src, n_free, tag):
            """sq_src: list of KD bf16 sq tiles [128, n_free] (already squared).
            Returns rinv16 [1, n_free] bf16 = rsqrt(mean + eps)."""
            ssq_ps = ps_row.tile([1, n_free], F32, tag="att_o")
            for d in range(KD):
                nc.tensor.matmul(ssq_ps, oneD_col, sq_src[d],
                                 start=(d == 0), stop=(d == KD - 1))
            return rms_finish(ssq_ps, n_free, tag, eps=True)

        def rms_finish(ssq_ps, n_free, tag, eps=False):
            # eps=1e-6 matters only at the embedding scale (layer-0 rms1);
            # everywhere else mean-sq >= ~0.2 and the add is skipped.
            src_ap = ssq_ps
            if eps:
                m_eps = rowp.tile([1, n_free], F32, tag="me")
                nc.vector.tensor_scalar_add(m_eps, ssq_ps, 1e-6)
                src_ap = m_eps
            rinv16 = rowp.tile([1, n_free], MMDT, tag="ri")
            nc.scalar.activation(out=rinv16, in_=src_ap, func=AF.Abs_reciprocal_sqrt)
            return rinv16

        def bcast16(rinv16, n_free, tag):
            """broadcast [1, n] bf16 row to [128, n] bf16 SBUF tile."""
            bc_ps = ps_mm.tile([128, n_free], F32, tag="mm512")
            nc.tensor.matmul(bc_ps, ones_row, rinv16, start=True, stop=True)
            bc = acts.tile([128, n_free], MMDT, tag=f"bc_{tag}")
            nc.scalar.copy(out=bc, in_=bc_ps)
            return bc

        # ---------- layer-0 rmsnorm stats (pipelined after casts) ----------
        sqs = []
        for d in range(KD):
            sq = tmp.tile([128, T], MMDT, tag="sq")
            nc.vector.tensor_mul(sq, h16[:, d, :], h16[:, d, :])
            sqs.append(sq)
        rinv16 = rms_stats(sqs, T, "rms1")

        # ---------- layers ----------
        for l in range(L):
            # ---- Q, K projections (read h16 directly) + rope (bf16, 4x DVE) ----
            qrot = acts.tile([128, KD, T], MMDT, tag="qrot")
            krot = acts.tile([128, KD, T], MMDT, tag="krot")
            cosL = acts.tile([128, T], MMDT, tag="cosL")
            sinL = acts.tile([128, T], MMDT, tag="sinL")
            rinv_col = persist.tile([128, TT], F32, tag=f"rcol{l}")

            def qk_group(w_d_, pair, pre=None):
                if pre is not None:
                    wg_e, wg_o = pre
                else:
                    wg_e = wpool.tile([128, KD, 128], dt_w, tag="wtile")
                    nc.sync.dma_start(out=wg_e, in_=w_d_[l, pair].transpose([1, 0, 2]))
                    wg_o = wpool.tile([128, KD, 128], dt_w, tag="wtile")
                    nc.sync.dma_start(out=wg_o, in_=w_d_[l, pair + 4].transpose([1, 0, 2]))
                ev_ps = ps_mm.tile([128, T], F32, tag="mm512")
                od_ps = ps_mm.tile([128, T], F32, tag="mm512")
                for kt in range(KD):
                    nc.tensor.matmul(ev_ps, wg_e[:, kt, :], h16[:, kt, :],
                                     start=(kt == 0), stop=(kt == KD - 1))
                for kt in range(KD):
                    nc.tensor.matmul(od_ps, wg_o[:, kt, :], h16[:, kt, :],
                                     start=(kt == 0), stop=(kt == KD - 1))
                return ev_ps, od_ps

            def rope_apply(rot, pair, ev_ps, od_ps):
                ev16 = eodp.tile([128, T], MMDT, tag="eod")
                od16 = eodp.tile([128, T], MMDT, tag="eod")
                nc.scalar.copy(out=ev16, in_=ev_ps)
                nc.scalar.copy(out=od16, in_=od_ps)
                t1 = tmp.tile([128, T], MMDT, tag="rtmp")
                t2 = tmp.tile([128, T], MMDT, tag="rtmp")
                nc.vector.tensor_mul(t1, ev16, cosL)
                nc.vector.tensor_mul(t2, od16, sinL)
                nc.vector.tensor_sub(rot[:, pair, :], t1, t2)
                t3 = tmp.tile([128, T], MMDT, tag="rtmp")
                t4 = tmp.tile([128, T], MMDT, tag="rtmp")
                nc.vector.tensor_mul(t3, ev16, sinL)
                nc.vector.tensor_mul(t4, od16, cosL)
                nc.vector.tensor_add(rot[:, pair + 4, :], t3, t4)

            # first q group goes ahead of the rinv-dependent table prep so the
            # PE never idles waiting on the rmsnorm chain
            ev0, od0 = qk_group(wq_d, 0, pre=pre_q)
            act_warm(AF.Exp, rinv16[0:1, 0:1])
            bc1 = bcast16(rinv16, T, "r1")
            nc.vector.tensor_mul(cosL, cos16, bc1)
            nc.vector.tensor_mul(sinL, sin16, bc1)
            for t in range(TT):
                rc_ps = ps_row.tile([128, 1], F32, tag="att_o")
                nc.tensor.matmul(rc_ps, rinv16[:, t * 128:(t + 1) * 128], ones11,
                                 start=True, stop=True)
                nc.vector.tensor_copy(out=rinv_col[:, t:t + 1], in_=rc_ps)
            rope_apply(qrot, 0, ev0, od0)
            for pair in range(4):
                for (w_d_, rot) in ((wq_d, qrot), (wk_d, krot)):
                    if pair == 0 and rot is qrot:
                        continue
                    ev_ps, od_ps = qk_group(w_d_, pair)
                    rope_apply(rot, pair, ev_ps, od_ps)

            # ---- V projections (read h16; rinv applied on the psum->sbuf copy).
            # Layer 0's V matmuls already ran in the prologue: only the rinv
            # scale fixup remains (in place).
            if l == 0:
                for t_v in range(TT):
                    for nh_v in range(2):
                        nc.scalar.activation(
                            out=v_sb[:, t_v, nh_v, :], in_=v_sb[:, t_v, nh_v, :],
                            func=AF.Copy, scale=rinv_col[:, t_v:t_v + 1])
            else:
                for t_v in range(TT):
                    for nh_v in range(2):
                        v_ps = ps_mm.tile([128, 512], F32, tag="mm512")
                        for kt in range(KD):
                            nc.tensor.matmul(
                                v_ps,
                                h16[:, kt, t_v * 128:(t_v + 1) * 128],
                                wv_sb[:, kt, nh_v, :],
                                start=(kt == 0), stop=(kt == KD - 1))
                        nc.scalar.activation(out=v_sb[:, t_v, nh_v, :], in_=v_ps,
                                             func=AF.Copy, scale=rinv_col[:, t_v:t_v + 1])

            # ---- attention (software-pipelined: den lags scores by 1 head,
            # dbc + attn@V lag by 2, so the PE never waits on the ACT/DVE
            # softmax chain) ----
            oT = acts.tile([128, KD, T], MMDT, tag="oT")
            e_tiles = {}
            den_tiles = {}
            dbc_tiles = {}

            def scores(h):
                me, mo, off = h // 2, 4 + h // 2, (h % 2) * 64
                s_ps = ps_att.tile([128, T], F32, tag="att_s")
                for t in range(TT):
                    ts_ = slice(t * 128, (t + 1) * 128)
                    nc.tensor.matmul(s_ps[:, ts_], krot[off:off + 64, me, ts_],
                                     qrot[off:off + 64, me, ts_],
                                     start=True, stop=False)
                    nc.tensor.matmul(s_ps[:, ts_], krot[off:off + 64, mo, ts_],
                                     qrot[off:off + 64, mo, ts_],
                                     start=False, stop=True)
                ef16 = tmp.tile([128, T], MMDT, tag="ef16")
                nc.scalar.activation(out=ef16, in_=s_ps, func=AF.Exp, scale=float(SCALE))
                em = epool.tile([128, T], MMDT, tag="e_mm")
                nc.vector.tensor_mul(em, ef16, mask_flat)
                e_tiles[h] = em

            def den(h):
                den_ps = ps_row.tile([1, T], F32, tag="att_o")
                nc.tensor.matmul(den_ps, ones_col, e_tiles[h], start=True, stop=True)
                dinv_f = dinvp.tile([1, T], F32, tag="dinv_f")
                nc.vector.reciprocal_approx_fast(out=dinv_f, in_=den_ps)
                dinv16 = dinvp.tile([1, T], MMDT, tag="dinv16")
                nc.vector.tensor_copy(out=dinv16, in_=dinv_f)
                den_tiles[h] = dinv16

            def attn_v(h):
                dbc_ps = ps_mm.tile([128, T], F32, tag="mm512")
                nc.tensor.matmul(dbc_ps, ones_row, den_tiles[h], start=True, stop=True)
                dbc16 = dbcp.tile([128, T], MMDT, tag="dbc16")
                nc.scalar.copy(out=dbc16, in_=dbc_ps)
                o_ps = ps_att.tile([128, T], F32, tag="att_o")
                for t in range(TT):
                    ts_ = slice(t * 128, (t + 1) * 128)
                    nc.tensor.matmul(
                        o_ps[:, ts_],
                        v_sb[:, t, h // 4, (h % 4) * 128:(h % 4 + 1) * 128],
                        e_tiles[h][:, ts_], start=True, stop=True)
                nc.vector.tensor_mul(oT[:, h, :], o_ps, dbc16)

            for h in range(H):
                scores(h)
                if h >= 2:
                    den(h - 2)
                if h >= 3:
                    attn_v(h - 3)
            den(H - 2)
            den(H - 1)
            attn_v(H - 3)
            attn_v(H - 2)
            attn_v(H - 1)
            act_warm(AF.Abs_reciprocal_sqrt, e_tiles[H - 1][0:1, 0:1])

            # ---- Wo + residual; rmsnorm2 stats interleaved (2-group lag) ----
            sq2s = []
            ssq2_ps = ps_row.tile([1, T], F32, tag="att_o")

            def wo_group(m):
                wg = wpool.tile([128, KD, 128], dt_w, tag="wtile")
                nc.sync.dma_start(out=wg, in_=wo_d[l, m].transpose([1, 0, 2]))
                wo_ps = ps_mm.tile([128, T], F32, tag="mm512")
                for kt in range(KD):
                    nc.tensor.matmul(wo_ps, wg[:, kt, :], oT[:, kt, :],
                                     start=(kt == 0), stop=(kt == KD - 1))
                nc.vector.tensor_add(hT[:, m, :], hT[:, m, :], wo_ps)
                if m < KD - 1:  # last cast deferred past the rms Sqrt (ACT order)
                    nc.scalar.copy(out=h16[:, m, :], in_=hT[:, m, :])
                sq = tmp.tile([128, T], MMDT, tag="sq")
                nc.vector.tensor_mul(sq, hT[:, m, :], hT[:, m, :])
                sq2s.append(sq)

            def ssq2_mm(d):
                nc.tensor.matmul(ssq2_ps, oneD_col, sq2s[d],
                                 start=(d == 0), stop=(d == KD - 1))

            for m in range(KD):
                wo_group(m)
                if m >= 2:
                    ssq2_mm(m - 2)
            ssq2_mm(KD - 2)
            ssq2_mm(KD - 1)
            rinv2 = rms_finish(ssq2_ps, T, "rms2")
            bc2 = bcast16(rinv2, T, "r2")
            act_warm(AF.Silu, bc2[0:1, 0:1])
            nc.scalar.copy(out=h16[:, KD - 1, :], in_=hT[:, KD - 1, :])

            # ---- MLP ----
            x2T = acts.tile([128, KD, T], MMDT, tag="x2T")
            for d in range(KD):
                nc.vector.tensor_mul(x2T[:, d, :], h16[:, d, :], bc2)
            y1 = acts.tile([128, KF, 512], MMDT, tag="y1")
            for j in range(KF):
                wg = wpool.tile([128, KD, 128], dt_w, tag="wtile")
                nc.sync.dma_start(out=wg, in_=w1_d[l, j].transpose([1, 0, 2]))
                y1_ps = ps_mm.tile([128, T], F32, tag="mm512")
                for kt in range(KD):
                    nc.tensor.matmul(y1_ps, wg[:, kt, :], x2T[:, kt, :],
                                     start=(kt == 0), stop=(kt == KD - 1))
                nc.scalar.activation(out=y1[:, j, :], in_=y1_ps, func=AF.Silu)
            act_warm(AF.Abs_reciprocal_sqrt, y1[0:1, KF - 1, 0:1])

            last_layer = (l == L - 1)
            # prefetch next layer's wv + first q-pair weights ahead of the W2
            # DMA storm so the next layer's QK phase starts without a DMA wait
            if not last_layer:
                for nh_ in range(2):
                    nc.sync.dma_start(out=wv_sb[:, :, nh_, :],
                                      in_=wv_d[l + 1, :, nh_].transpose([1, 0, 2]))
                pre_q = prefetch_wq(l + 1)
            sq3s = []
            ssq3_ps = ps_row.tile([1, T if not last_layer else MSL], F32, tag="att_o")
            exts = []

            def w2_group(m):
                wg2 = w2pool.tile([128, KF, 128], dt_w, tag="w2tile")
                for half in range(2):
                    hs = slice(half * (KF // 2), (half + 1) * (KF // 2))
                    nc.sync.dma_start(out=wg2[:, hs, :],
                                      in_=w2_d[l, m, hs].transpose([1, 0, 2]))
                y2_ps = ps_mm.tile([128, T], F32, tag="mm512")
                for j in range(KF):
                    nc.tensor.matmul(y2_ps, wg2[:, j, :], y1[:, j, :],
                                     start=(j == 0), stop=(j == KF - 1))
                nc.vector.tensor_add(hT[:, m, :], hT[:, m, :], y2_ps)
                if not last_layer:
                    nc.scalar.copy(out=h16[:, m, :], in_=hT[:, m, :])
                    sq = tmp.tile([128, T], MMDT, tag="sq")
                    nc.vector.tensor_mul(sq, hT[:, m, :], hT[:, m, :])
                    sq3s.append(sq)
                else:
                    # extract last-token columns, square them (final norm stats)
                    ext = persist.tile([128, MSL], F32, tag=f"ext{m}")
                    src = hT[:, m, :].rearrange("p (e w) -> p e w", w=EVLEN)[:, :, EVLEN - 1]
                    nc.vector.tensor_copy(out=ext, in_=src)
                    exts.append(ext)
                    sq = tmp.tile([128, MSL], MMDT, tag="sqe")
                    nc.vector.tensor_mul(sq, ext, ext)
                    sq3s.append(sq)

            def ssq3_mm(d):
                nc.tensor.matmul(ssq3_ps, oneD_col, sq3s[d],
                                 start=(d == 0), stop=(d == KD - 1))

            for m in range(KD):
                w2_group(m)
                if m >= 2:
                    ssq3_mm(m - 2)
            ssq3_mm(KD - 2)
            ssq3_mm(KD - 1)
            if not last_layer:
                rinv16 = rms_finish(ssq3_ps, T, "rms1")

        # ---------- final norm on extracted columns ----------
        rinvf = rms_finish(ssq3_ps, MSL, "rmsf")
        bcf_ps = ps_mm.tile([128, MSL], F32, tag="mm512")
        nc.tensor.matmul(bcf_ps, ones_row, rinvf, start=True, stop=True)
        for d in range(KD):
            outT = dinvp.tile([128, MSL], F32, tag="outT")
            nc.vector.scalar_tensor_tensor(
                out=outT, in0=exts[d], scalar=lnf_sb[:, d:d + 1], in1=bcf_ps,
                op0=ALU.mult, op1=ALU.mult)
            nc.sync.dma_start(out=out_d[d], in_=outT)

    nc.compile()
    return nc


# =============================================================
# host side
# =============================================================

def _qperm():
    r = np.arange(512)
    h, j2 = r // 64, r % 64
    return np.concatenate([h * 128 + 2 * j2, h * 128 + 2 * j2 + 1])


def prep_inputs(inputs):
    """Build the per-core in_maps (host-side layout/preprocessing only)."""
    ids = np.ascontiguousarray(inputs["input_ids"]).astype(np.int32)
    pos = np.ascontiguousarray(inputs["position_ids"]).astype(np.int32)
    svl = np.ascontiguousarray(inputs["seq_varlen"]).astype(np.int64)
    emb = np.ascontiguousarray(inputs["emb"], dtype=np.float32)
    ln1, ln2, lnf = inputs["ln1"], inputs["ln2"], inputs["lnf"]

    cum = np.cumsum(svl)
    assert cum[-1] == S, "kernel assumes packed tokens fill S exactly"
    seg = np.searchsorted(cum, np.arange(S), side="right")
    for c in range(1, NCORES):
        assert seg[c * T - 1] != seg[c * T], "segment straddles core boundary"
    last_idx = cum - 1
    for c in range(NCORES):
        li = last_idx[c * MSL:(c + 1) * MSL] - c * T
        assert np.array_equal(li, EVLEN - 1 + EVLEN * np.arange(MSL)), \
            "kernel assumes fixed EVLEN segments"

    qperm = _qperm()
    wq = np.empty((L, KD, KD, 128, 128), MMNP)
    wk = np.empty((L, KD, KD, 128, 128), MMNP)
    wv = np.empty((L, KD, 2, 128, 512), MMNP)
    wo = np.empty((L, KD, KD, 128, 128), MMNP)
    w1 = np.empty((L, KF, KD, 128, 128), MMNP)
    w2 = np.empty((L, KD, KF, 128, 128), MMNP)
    for l in range(L):
        g1 = ln1[l][:, None].astype(np.float32)
        g2 = ln2[l][:, None].astype(np.float32)
        Wq_p = (g1 * inputs["Wq"][l])[:, qperm]
        Wk_p = (g1 * inputs["Wk"][l])[:, qperm]
        Wv_p = g1 * inputs["Wv"][l]
        W1_p = g2 * inputs["W1"][l]
        wq[l] = Wq_p.reshape(KD, 128, KD, 128).transpose(2, 0, 1, 3).astype(MMNP)
        wk[l] = Wk_p.reshape(KD, 128, KD, 128).transpose(2, 0, 1, 3).astype(MMNP)
        wv[l] = Wv_p.reshape(KD, 128, 2, 512).transpose(0, 2, 1, 3).astype(MMNP)
        wo[l] = np.asarray(inputs["Wo"][l]).reshape(KD, 128, KD, 128).transpose(2, 0, 1, 3).astype(MMNP)
        w1[l] = W1_p.reshape(KD, 128, KF, 128).transpose(2, 0, 1, 3).astype(MMNP)
        w2[l] = np.asarray(inputs["W2"][l]).reshape(KF, 128, KD, 128).transpose(2, 0, 1, 3).astype(MMNP)

    invf = (1.0 / (ROPE_BASE ** (np.arange(0, DH, 2, dtype=np.float32) / DH)))
    invf2 = np.tile(invf, 2)[:, None].astype(np.float32)
    lnft = np.asarray(lnf, dtype=np.float32).reshape(KD, 128)

    in_maps = []
    for c in range(NCORES):
        sl = slice(c * T, (c + 1) * T)
        seg_c = seg[sl]
        maskT = np.empty((TT, 128, 128), MMNP)
        for t in range(TT):
            sg = seg_c[t * 128:(t + 1) * 128]
            same = (sg[:, None] == sg[None, :])
            kq = np.arange(128)
            maskT[t] = (same & (kq[:, None] <= kq[None, :])).astype(MMNP)
        ang = invf2 * pos[sl].astype(np.float32)[None, :]
        m = {
            "emb": emb,
            "idsc": ids[sl].reshape(TT, 128, 1),
            "maskT": maskT,
            "lnft": lnft,
            "wq": wq, "wk": wk, "wv": wv, "wo": wo, "w1": w1, "w2": w2,
            "costab": np.cos(ang).astype(MMNP),
            "sintab": np.sin(ang).astype(MMNP),
        }
        in_maps.append(m)
    return in_maps


def assemble_output(results):
    """results: list of per-core dicts with 'out' [KD, 128, MSL] -> [8, 16, D]."""
    out = np.empty((NCORES, MSL, D), np.float32)
    for c in range(NCORES):
        a = results[c]["out"]  # [KD, 128, MSL]
        out[c] = a.transpose(2, 0, 1).reshape(MSL, D)
    return out.reshape(NCORES, MSL, D)


_CACHE = {}


def kernel(**inputs) -> np.ndarray:
    from concourse.bass_utils import run_bass_kernel_spmd
    inputs = {k: np.asarray(v) for k, v in inputs.items()}
    if "nc" not in _CACHE:
        _CACHE["nc"] = build_program()
    nc = _CACHE["nc"]
    in_maps = prep_inputs(inputs)
    res = run_bass_kernel_spmd(nc, in_maps, core_ids=list(range(NCORES)))
    return assemble_output(res.results)
